# revision 27
# baseline (speedup 1.0000x reference)
"""Trainium2 Bass kernel for token-level contrastive loss (CLIP-style with
softmax token pooling), distributed over 8 NeuronCores.

Strategy: shard the token axis T (196 -> padded 200 = 8 cores x 25 slices).
Each core computes, for its local t-slices, the full [B, B] partial softmax-
pooling sums S = sum_t exp(cos_t) and V = sum_t cos_t*exp(cos_t); these are
AllReduced, then every core redundantly computes the scalar bidirectional
InfoNCE loss (core 0's output is returned).

v2 pipeline: the host ships bf16 tokens already transposed to [d, t, b]
layout, so the device does no transposes at all.  Per core:
  - token norms: DVE squares + ones-column matmuls that land n2 in [t, b]
    PSUM tiles; rsqrt via Ln/Exp on ACT
  - r_t (text rsqrt) transposed to [b-partition, t] by tiny PE transposes,
    consumed directly from PSUM as the ACT exp scale
  - visual tokens normalized in-place by GpSimd (partition_broadcast of the
    rsqrt row + multiply), keeping DVE free
  - per (b-tile, t): bf16 matmul -> dots PSUM; exp on ScalarE with r_t
    folded into the activation scale; cos*e via one fused DVE stt; S and V
    accumulated across all 25 t in dedicated PSUM banks by identity-matmuls
  - S/V flushed once per b-tile, AllReduced, and the scalar loss computed
    redundantly on every core.
"""

import sys

sys.path.insert(0, "/opt/trn_rl_repo")

import numpy as np

import concourse.bass as bass
import concourse.mybir as mybir
import concourse.tile as tile
from concourse import bacc
from concourse.bass import ds, ts
from concourse.bass_utils import run_bass_kernel_spmd
from concourse.masks import make_identity

B = 512
T = 196
D = 256
NCORES = 8
TPAD = 200
TLOC = TPAD // NCORES  # 25
G = 5                  # t-slices per norm group
NG = TLOC // G         # 5 groups
NB = B // 128          # 4 b-tiles
NPAD = TPAD - T        # 4 zero pad slices globally
TEMP = 0.07

F32 = mybir.dt.float32
F16 = mybir.dt.float16
BF16 = mybir.dt.bfloat16


def _build_program():
    nc = bacc.Bacc(
        "TRN2",
        target_bir_lowering=False,
        debug=False,
        num_devices=NCORES,
    )
    # host-pretransposed: [p=d%128, t, h=d//128, b]
    text_in = nc.dram_tensor("text", [128, TLOC, 2, B], BF16, kind="ExternalInput")
    vis_in = nc.dram_tensor("vis", [128, TLOC, 2, B], BF16, kind="ExternalInput")
    out = nc.dram_tensor("out", [1, 1], F32, kind="ExternalOutput")

    # n2 rows are bank-packed by PE at partition bases {0,32,64}; triples of
    # tokens per PSUM tile, groups of 5 -> sub-triples (3, 2) per group.
    TRIPLES = ((0, 3), (3, 2))  # (start_tl, count) within a group

    with tile.TileContext(nc) as tc:
        with (
            tc.tile_pool(name="const", bufs=1) as cpool,
            tc.tile_pool(name="tok", bufs=1) as tokpool,
            tc.tile_pool(name="xsq", bufs=1) as xsqpool,
            tc.tile_pool(name="rsb", bufs=1) as rpool,
            tc.tile_pool(name="et", bufs=3) as epool,
            tc.tile_pool(name="acc", bufs=1) as accpool,
            tc.tile_pool(name="fin", bufs=1) as finpool,
            tc.tile_pool(name="rvp", bufs=2) as rvpool,
            tc.tile_pool(name="scr", bufs=1) as scrpool,
            tc.tile_pool(name="psn", bufs=1, space="PSUM") as ps_n2,
            tc.tile_pool(name="psr", bufs=1, space="PSUM") as ps_rt,
            tc.tile_pool(name="pssv", bufs=2, space="PSUM") as ps_sv,
            tc.tile_pool(name="psd", bufs=2, space="PSUM") as ps_dots,
            tc.tile_pool(name="dram", bufs=1, space="DRAM") as dpool,
        ):
            # ---- constants ----
            ident = cpool.tile([128, 128], BF16, tag="ident")
            make_identity(nc, ident[:])
            identf = cpool.tile([128, 128], F32, tag="identf")
            make_identity(nc, identf[:])
            ones_bf = cpool.tile([128, 1], BF16, tag="onesbf")
            nc.gpsimd.memset(ones_bf[:], 1.0)
            ones = cpool.tile([128, 1], F32, tag="ones")
            nc.gpsimd.memset(ones[:], 1.0)
            eps_b = cpool.tile([128, 1], F32, tag="epsb")
            nc.gpsimd.memset(eps_b[:], 1e-12)
            diag_mask = cpool.tile([128, NB, 512], BF16, tag="dmask")
            nc.gpsimd.memset(diag_mask[:], 0.0)
            nc.gpsimd.affine_select(
                out=diag_mask[:],
                in_=diag_mask[:],
                compare_op=mybir.AluOpType.not_equal,
                fill=1.0,
                base=0,
                pattern=[[-128, NB], [1, 512]],
                channel_multiplier=-1,
            )

            # ---- persistent SBUF tiles ----
            texT = tokpool.tile([128, TLOC, 2, B], BF16, tag="texT")
            visT = tokpool.tile([128, TLOC, 2, B], BF16, tag="visT")
            # n2 / r gathered as [tl(5), g, (text|vis), b]
            r_t_sb = rpool.tile([128, NB, TLOC], F32, tag="rtsb")
            S_sb = accpool.tile([128, NB, 512], F16, tag="S")
            V_sb = accpool.tile([128, NB, 512], F16, tag="V")

            # ---- input loads: all issued up-front on the sync queue so the
            # DMA engines stream continuously (nothing queued behind them) ----
            for g in range(NG):
                tg = ds(g * G, G)
                nc.sync.dma_start(texT[:, tg, :, :], text_in.ap()[:, tg, :, :])
                nc.sync.dma_start(visT[:, tg, :, :], vis_in.ap()[:, tg, :, :])

            # ---- phase A helper: norms, rsqrt, vis normalize for group g ----
            def emit_A(g):
                tg = ds(g * G, G)
                xsq = xsqpool.tile([128, G, 2, 2, 512], BF16, tag="xsq")
                nc.vector.tensor_mul(
                    xsq[:, :, :, 0, :], texT[:, tg, :, :], texT[:, tg, :, :]
                )
                nc.gpsimd.tensor_mul(
                    xsq[:, :, :, 1, :], visT[:, tg, :, :], visT[:, tg, :, :]
                )

                # n2 rows -> PSUM, bank-packed 3 tokens at bases {0,32,64};
                # evict to SBUF (partition-preserving), then SBUF->SBUF DMAs
                # scatter rows onto [tl] partitions of n2g.
                n2g = scrpool.tile([G, 2, B], F32, tag="n2g")
                for tens in range(2):
                    for jstart, jcount in TRIPLES:
                        x_ps = ps_n2.tile([65, 512], F32, tag="xps")
                        for s in range(jcount):
                            tl = jstart + s
                            for h in range(2):
                                nc.tensor.matmul(
                                    x_ps[ds(32 * s, 1), :],
                                    ones_bf[:],
                                    xsq[:, tl, h, tens, :],
                                    start=(h == 0), stop=(h == 1),
                                    skip_group_check=True,
                                )
                        x_sb = xsqpool.tile([65, 512], F32, tag="xsb")
                        nc.vector.tensor_copy(
                            x_sb[ds(0, 32 * (jcount - 1) + 1), :],
                            x_ps[ds(0, 32 * (jcount - 1) + 1), :],
                        )
                        nc.scalar.dma_start(
                            n2g[ds(jstart, jcount), tens, :],
                            x_sb[ds(0, jcount, 32), :],
                        )

                # rsqrt: r = exp(-0.5*ln(n2+eps)); text half f32, vis bf16
                lnscr = scrpool.tile([G, 2, B], F32, tag="lnscr")
                nc.scalar.activation(
                    lnscr[:], n2g[:],
                    mybir.ActivationFunctionType.Ln, bias=eps_b[ds(0, G)],
                )
                r_g = rvpool.tile([G, B], F32, tag="rg")
                nc.scalar.activation(
                    r_g[:], lnscr[:, 0, :],
                    mybir.ActivationFunctionType.Exp, scale=-0.5,
                )
                r_gv = rvpool.tile([G, B], BF16, tag="rgv")
                nc.scalar.activation(
                    r_gv[:], lnscr[:, 1, :],
                    mybir.ActivationFunctionType.Exp, scale=-0.5,
                )

                # r_t -> [b-partition, (i, t)] via PE transposes (f32)
                rt_ps = ps_rt.tile([128, NB, G], F32, tag="rtps")
                for i in range(NB):
                    nc.tensor.matmul(
                        rt_ps[:, i, :],
                        r_g[:, ts(i, 128)],
                        identf[ds(0, G), ds(0, G)],
                        is_transpose=True,
                        skip_group_check=True,
                    )
                nc.vector.tensor_copy(r_t_sb[:, :, ds(g * G, G)], rt_ps[:])

                # vis r rows to partition 0 (partition_broadcast reads p0),
                # then per-token broadcast on GpSimd + normalize on DVE
                rv_flat = scrpool.tile([1, G, B], BF16, tag="rvflat")
                nc.scalar.dma_start(rv_flat[:], r_gv[:])
                for tl in range(G):
                    t = g * G + tl
                    rv_bc = rvpool.tile([128, B], BF16, tag="rvbc")
                    nc.gpsimd.partition_broadcast(
                        rv_bc[:], rv_flat[:, tl, :]
                    )
                    nc.vector.tensor_mul(
                        visT[:, t, 0, :], visT[:, t, 0, :], rv_bc[:]
                    )
                    nc.vector.tensor_mul(
                        visT[:, t, 1, :], visT[:, t, 1, :], rv_bc[:]
                    )

            # ---- phase B: dots, exp, cos*e, S/V PSUM accumulation ----
            # i = 0 is interleaved with phase A group emission; groups feed
            # the pipeline just-in-time.  Software pipeline depth 2.
            def make_phase_b(i):
                S_ps = ps_sv.tile([128, 512], F32, tag="Sps")
                V_ps = ps_sv.tile([128, 512], F32, tag="Vps")
                state = {"S": S_ps, "V": V_ps, "dots": {}}

                def emit_dots(t):
                    dots = ps_dots.tile([128, 512], F32, tag="dots")
                    nc.tensor.matmul(
                        dots[:], texT[:, t, 0, ts(i, 128)], visT[:, t, 0, :],
                        start=True, stop=False,
                    )
                    nc.tensor.matmul(
                        dots[:], texT[:, t, 1, ts(i, 128)], visT[:, t, 1, :],
                        start=False, stop=True,
                    )
                    state["dots"][t] = dots

                def emit_tail(t):
                    dots = state["dots"].pop(t)
                    e_t = epool.tile([128, 512], BF16, tag="e")
                    nc.scalar.activation(
                        e_t[:], dots[:],
                        mybir.ActivationFunctionType.Exp,
                        scale=r_t_sb[:, i, ds(t, 1)],
                    )
                    tmp_t = epool.tile([128, 512], BF16, tag="tmp")
                    nc.vector.scalar_tensor_tensor(
                        out=tmp_t[:],
                        in0=dots[:],
                        scalar=r_t_sb[:, i, ds(t, 1)],
                        in1=e_t[:],
                        op0=mybir.AluOpType.mult,
                        op1=mybir.AluOpType.mult,
                    )
                    nc.tensor.matmul(
                        state["S"][:], ident[:], e_t[:],
                        start=(t == 0), stop=(t == TLOC - 1),
                        skip_group_check=True,
                    )
                    nc.tensor.matmul(
                        state["V"][:], ident[:], tmp_t[:],
                        start=(t == 0), stop=(t == TLOC - 1),
                        skip_group_check=True,
                    )

                def finish(i=i):
                    nc.vector.tensor_copy(S_sb[:, i, :], state["S"][:])
                    nc.vector.tensor_copy(V_sb[:, i, :], state["V"][:])

                return emit_dots, emit_tail, finish

            DEPTH = 1
            streams = {}

            def b_steps(i, gg):
                """Emit group gg's 5 pipeline steps for b-tile i, fully
                drained (dots pool has only 2 banks)."""
                if i not in streams:
                    streams[i] = make_phase_b(i)
                emit_dots_i, emit_tail_i, finish_i = streams[i]
                pend = []
                for tl in range(G):
                    t = gg * G + tl
                    emit_dots_i(t)
                    pend.append(t)
                    if len(pend) > DEPTH:
                        emit_tail_i(pend.pop(0))
                while pend:
                    emit_tail_i(pend.pop(0))

            def finish_b(i):
                streams[i][2]()

            # phase A groups feed b-tiles 0 and 1, lagging 1 and 2 groups
            emit_A(0)
            emit_A(1); b_steps(0, 0)
            emit_A(2); b_steps(0, 1); b_steps(1, 0)
            emit_A(3); b_steps(0, 2); b_steps(1, 1)
            emit_A(4); b_steps(0, 3); b_steps(1, 2)
            b_steps(0, 4); b_steps(1, 3)
            finish_b(0)
            b_steps(1, 4)
            finish_b(1)

            def run_stream_i(i):
                for gg in range(NG):
                    b_steps(i, gg)
                finish_b(i)

            # ---- AllReduce S/V in two fp16 chunks (i-pairs), overlapping the
            # first chunk's collective with the second half of phase B ----
            cc_in = dpool.tile([2, 2, 128, 2 * 512], F16, tag="cc_in")
            cc_out0 = dpool.tile(
                [2, 128, 2 * 512], F16, tag="cc_out0", addr_space="Shared"
            )
            cc_out1 = dpool.tile(
                [2, 128, 2 * 512], F16, tag="cc_out1", addr_space="Shared"
            )
            cc_outs = (cc_out0, cc_out1)

            def ar_chunk(ch):
                isl = ds(2 * ch, 2)
                nc.sync.dma_start(
                    cc_in[ch, 0], S_sb[:, isl, :].rearrange("p i c -> p (i c)")
                )
                nc.sync.dma_start(
                    cc_in[ch, 1], V_sb[:, isl, :].rearrange("p i c -> p (i c)")
                )
                nc.gpsimd.collective_compute(
                    "AllReduce",
                    mybir.AluOpType.add,
                    replica_groups=[list(range(NCORES))],
                    ins=[cc_in[ch].opt()],
                    outs=[cc_outs[ch][:].opt()],
                )
                nc.sync.dma_start(
                    S_sb[:, isl, :].rearrange("p i c -> p (i c)"), cc_outs[ch][0]
                )
                nc.sync.dma_start(
                    V_sb[:, isl, :].rearrange("p i c -> p (i c)"), cc_outs[ch][1]
                )

            ar_chunk(0)
            run_stream_i(2)
            run_stream_i(3)
            ar_chunk(1)

            # ---- final scalar loss, split per AR chunk so chunk-0 finals
            # overlap the chunk-1 collective ----
            scr2 = finpool.tile([128, NB, 512], F32, tag="scr")
            sim = V_sb  # sim = V/S computed in place over V
            diag_p = finpool.tile([128, 2], F32, tag="diagp")
            rowsum = finpool.tile([128, NB], F32, tag="rowsum")
            col_ps = ps_sv.tile([1, 512], F32, tag="Sps")

            def finals_chunk(ch):
                isl = ds(2 * ch, 2)
                # pad correction: each global pad slice added exp(0)=1 to S
                nc.vector.tensor_scalar_add(
                    S_sb[:, isl, :], S_sb[:, isl, :], float(-NPAD)
                )
                nc.scalar.activation(
                    scr2[:, isl, :], S_sb[:, isl, :],
                    mybir.ActivationFunctionType.Ln,
                )
                nc.scalar.activation(
                    scr2[:, isl, :], scr2[:, isl, :],
                    mybir.ActivationFunctionType.Exp, scale=-1.0,
                )
                nc.vector.tensor_mul(
                    sim[:, isl, :], V_sb[:, isl, :], scr2[:, isl, :]
                )
                nc.vector.scalar_tensor_tensor(
                    out=scr2[:, isl, :],
                    in0=sim[:, isl, :],
                    scalar=1.0,
                    in1=diag_mask[:, isl, :],
                    op0=mybir.AluOpType.mult,
                    op1=mybir.AluOpType.mult,
                    accum_out=diag_p[:, ds(ch, 1)],
                )
                for i in range(2 * ch, 2 * ch + 2):
                    nc.scalar.activation(
                        scr2[:, i, :], sim[:, i, :],
                        mybir.ActivationFunctionType.Exp,
                        scale=1.0 / TEMP,
                        accum_out=rowsum[:, ds(i, 1)],
                    )
                for i in range(2 * ch, 2 * ch + 2):
                    nc.tensor.matmul(
                        col_ps[:], ones[:], scr2[:, i, :],
                        start=(i == 0), stop=(i == NB - 1),
                        skip_group_check=True,
                    )

            finals_chunk(0)
            finals_chunk(1)

            lse_row = finpool.tile([128, NB], F32, tag="lserow")
            nc.scalar.activation(
                lse_row[:], rowsum[:], mybir.ActivationFunctionType.Ln,
            )
            lse_col = finpool.tile([1, 512], F32, tag="lsecol")
            nc.scalar.activation(
                lse_col[:], col_ps[:], mybir.ActivationFunctionType.Ln,
            )
            csum = finpool.tile([1, 1], F32, tag="csum")
            nc.vector.reduce_sum(csum[:], lse_col[:], axis=mybir.AxisListType.X)
            red_ps = ps_sv.tile([1, 2], F32, tag="Vps")
            lse_row_red = finpool.tile([128, 1], F32, tag="lserr")
            nc.vector.reduce_sum(
                lse_row_red[:], lse_row[:], axis=mybir.AxisListType.X
            )
            diag_r = finpool.tile([128, 1], F32, tag="diagr")
            nc.vector.reduce_sum(diag_r[:], diag_p[:], axis=mybir.AxisListType.X)
            nc.tensor.matmul(
                red_ps[:, 0:1], ones[:], lse_row_red[:], start=True, stop=True
            )
            nc.tensor.matmul(
                red_ps[:, 1:2], ones[:], diag_r[:], start=True, stop=True
            )
            red_sb = finpool.tile([1, 2], F32, tag="redsb")
            nc.vector.tensor_copy(red_sb[:], red_ps[:])
            t_a = finpool.tile([1, 1], F32, tag="ta")
            nc.vector.tensor_add(t_a[:], red_sb[:, 0:1], csum[:])
            nc.vector.tensor_scalar_mul(t_a[:], t_a[:], 0.5 / B)
            t_b = finpool.tile([1, 1], F32, tag="tb")
            nc.vector.tensor_scalar_mul(
                t_b[:], red_sb[:, 1:2], 1.0 / (B * TEMP)
            )
            loss_t = finpool.tile([1, 1], F32, tag="loss")
            nc.vector.tensor_sub(loss_t[:], t_a[:], t_b[:])
            nc.sync.dma_start(out.ap(), loss_t[:])

    nc.compile()
    return nc


_CACHE = {}


def _get_program():
    if "nc" not in _CACHE:
        _CACHE["nc"] = _build_program()
    return _CACHE["nc"]


def _prep_core_inputs(text: np.ndarray, vis: np.ndarray):
    """Pad T, cast bf16, transpose to [p=d%128, t, h=d//128, b] per core."""
    import ml_dtypes

    bf16 = ml_dtypes.bfloat16
    tp = np.zeros((B, TPAD, D), np.float32)
    vp = np.zeros((B, TPAD, D), np.float32)
    tp[:, :T] = text
    vp[:, :T] = vis
    tpb = tp.astype(bf16)
    vpb = vp.astype(bf16)

    in_maps = []
    for k in range(NCORES):
        sl = slice(k * TLOC, (k + 1) * TLOC)
        core = {}
        for name, arr in (("text", tpb[:, sl]), ("vis", vpb[:, sl])):
            # [b, t, d] -> [d, t, b] -> [h, p, t, b] -> [p, t, h, b]
            x = arr.transpose(2, 1, 0).reshape(2, 128, TLOC, B)
            core[name] = np.ascontiguousarray(x.transpose(1, 2, 0, 3))
        in_maps.append(core)
    return in_maps


def kernel(text_tokens: np.ndarray, visual_tokens: np.ndarray) -> np.ndarray:
    text = np.ascontiguousarray(np.asarray(text_tokens, dtype=np.float32))
    vis = np.ascontiguousarray(np.asarray(visual_tokens, dtype=np.float32))
    assert text.shape == (B, T, D) and vis.shape == (B, T, D)

    in_maps = _prep_core_inputs(text, vis)
    nc = _get_program()
    res = run_bass_kernel_spmd(nc, in_maps, core_ids=list(range(NCORES)))
    loss = np.float32(res.results[0]["out"].reshape(-1)[0])
    return np.asarray(loss, dtype=np.float32).reshape(())


# revision 28
# speedup vs baseline: 1.0907x; 1.0907x over previous
"""Trainium2 Bass kernel for token-level contrastive loss (CLIP-style with
softmax token pooling), distributed over 8 NeuronCores.

Strategy: shard the token axis T (196 -> padded 200 = 8 cores x 25 slices).
Each core computes, for its local t-slices, the full [B, B] partial softmax-
pooling sums S = sum_t exp(cos_t) and V = sum_t cos_t*exp(cos_t); these are
AllReduced, then every core redundantly computes the scalar bidirectional
InfoNCE loss (core 0's output is returned).

v2 pipeline: the host ships bf16 tokens already transposed to [d, t, b]
layout, so the device does no transposes at all.  Per core:
  - token norms: DVE squares + ones-column matmuls that land n2 in [t, b]
    PSUM tiles; rsqrt via Ln/Exp on ACT
  - r_t (text rsqrt) transposed to [b-partition, t] by tiny PE transposes,
    consumed directly from PSUM as the ACT exp scale
  - visual tokens normalized in-place by GpSimd (partition_broadcast of the
    rsqrt row + multiply), keeping DVE free
  - per (b-tile, t): bf16 matmul -> dots PSUM; exp on ScalarE with r_t
    folded into the activation scale; cos*e via one fused DVE stt; S and V
    accumulated across all 25 t in dedicated PSUM banks by identity-matmuls
  - S/V flushed once per b-tile, AllReduced, and the scalar loss computed
    redundantly on every core.
"""

import sys

sys.path.insert(0, "/opt/trn_rl_repo")

import numpy as np

import concourse.bass as bass
import concourse.mybir as mybir
import concourse.tile as tile
from concourse import bacc
from concourse.bass import ds, ts
from concourse.bass_utils import run_bass_kernel_spmd
from concourse.masks import make_identity

B = 512
T = 196
D = 256
NCORES = 8
TPAD = 200
TLOC = TPAD // NCORES  # 25
G = 5                  # t-slices per norm group
NG = TLOC // G         # 5 groups
NB = B // 128          # 4 b-tiles
NPAD = TPAD - T        # 4 zero pad slices globally
TEMP = 0.07

F32 = mybir.dt.float32
F16 = mybir.dt.float16
BF16 = mybir.dt.bfloat16


def _build_program():
    nc = bacc.Bacc(
        "TRN2",
        target_bir_lowering=False,
        debug=False,
        num_devices=NCORES,
    )
    # host-pretransposed: [p=d%128, t, h=d//128, b]
    text_in = nc.dram_tensor("text", [128, TLOC, 2, B], BF16, kind="ExternalInput")
    vis_in = nc.dram_tensor("vis", [128, TLOC, 2, B], BF16, kind="ExternalInput")
    out = nc.dram_tensor("out", [1, 1], F32, kind="ExternalOutput")

    # n2 rows are bank-packed by PE at partition bases {0,32,64}; triples of
    # tokens per PSUM tile, groups of 5 -> sub-triples (3, 2) per group.
    TRIPLES = ((0, 3), (3, 2))  # (start_tl, count) within a group

    with tile.TileContext(nc) as tc:
        with (
            tc.tile_pool(name="const", bufs=1) as cpool,
            tc.tile_pool(name="tok", bufs=1) as tokpool,
            tc.tile_pool(name="xsq", bufs=1) as xsqpool,
            tc.tile_pool(name="rsb", bufs=1) as rpool,
            tc.tile_pool(name="et", bufs=3) as epool,
            tc.tile_pool(name="acc", bufs=1) as accpool,
            tc.tile_pool(name="fin", bufs=1) as finpool,
            tc.tile_pool(name="rvp", bufs=2) as rvpool,
            tc.tile_pool(name="scr", bufs=1) as scrpool,
            tc.tile_pool(name="psn", bufs=1, space="PSUM") as ps_n2,
            tc.tile_pool(name="psr", bufs=1, space="PSUM") as ps_rt,
            tc.tile_pool(name="pssv", bufs=2, space="PSUM") as ps_sv,
            tc.tile_pool(name="psd", bufs=2, space="PSUM") as ps_dots,
            tc.tile_pool(name="dram", bufs=1, space="DRAM") as dpool,
        ):
            # ---- constants ----
            ident = cpool.tile([128, 128], BF16, tag="ident")
            make_identity(nc, ident[:])
            identf = cpool.tile([128, 128], F32, tag="identf")
            make_identity(nc, identf[:])
            ones_bf = cpool.tile([128, 1], BF16, tag="onesbf")
            nc.gpsimd.memset(ones_bf[:], 1.0)
            ones = cpool.tile([128, 1], F32, tag="ones")
            nc.gpsimd.memset(ones[:], 1.0)
            eps_b = cpool.tile([128, 1], F32, tag="epsb")
            nc.gpsimd.memset(eps_b[:], 1e-12)
            diag_mask = cpool.tile([128, NB, 512], BF16, tag="dmask")
            nc.gpsimd.memset(diag_mask[:], 0.0)
            nc.gpsimd.affine_select(
                out=diag_mask[:],
                in_=diag_mask[:],
                compare_op=mybir.AluOpType.not_equal,
                fill=1.0,
                base=0,
                pattern=[[-128, NB], [1, 512]],
                channel_multiplier=-1,
            )

            # ---- persistent SBUF tiles ----
            texT = tokpool.tile([128, TLOC, 2, B], BF16, tag="texT")
            visT = tokpool.tile([128, TLOC, 2, B], BF16, tag="visT")
            # n2 / r gathered as [tl(5), g, (text|vis), b]
            r_t_sb = rpool.tile([128, NB, TLOC], F32, tag="rtsb")
            S_sb = accpool.tile([128, NB, 512], F16, tag="S")
            V_sb = accpool.tile([128, NB, 512], F16, tag="V")

            # ---- input loads: all issued up-front on the sync queue so the
            # DMA engines stream continuously (nothing queued behind them) ----
            for g in range(NG):
                tg = ds(g * G, G)
                nc.sync.dma_start(texT[:, tg, :, :], text_in.ap()[:, tg, :, :])
                nc.sync.dma_start(visT[:, tg, :, :], vis_in.ap()[:, tg, :, :])

            # ---- phase A helper: norms, rsqrt, vis normalize for group g ----
            def emit_A(g):
                tg = ds(g * G, G)
                xsq = xsqpool.tile([128, G, 2, 2, 512], BF16, tag="xsq")
                nc.vector.tensor_mul(
                    xsq[:, :, :, 0, :], texT[:, tg, :, :], texT[:, tg, :, :]
                )
                nc.vector.tensor_mul(
                    xsq[:, :, :, 1, :], visT[:, tg, :, :], visT[:, tg, :, :]
                )

                # n2 rows -> PSUM, bank-packed 3 tokens at bases {0,32,64};
                # evict to SBUF (partition-preserving), then SBUF->SBUF DMAs
                # scatter rows onto [tl] partitions of n2g.
                n2g = scrpool.tile([G, 2, B], F32, tag="n2g")
                for tens in range(2):
                    for jstart, jcount in TRIPLES:
                        x_ps = ps_n2.tile([65, 512], F32, tag="xps")
                        for s in range(jcount):
                            tl = jstart + s
                            for h in range(2):
                                nc.tensor.matmul(
                                    x_ps[ds(32 * s, 1), :],
                                    ones_bf[:],
                                    xsq[:, tl, h, tens, :],
                                    start=(h == 0), stop=(h == 1),
                                    skip_group_check=True,
                                )
                        x_sb = xsqpool.tile([65, 512], F32, tag="xsb")
                        nc.vector.tensor_copy(
                            x_sb[ds(0, 32 * (jcount - 1) + 1), :],
                            x_ps[ds(0, 32 * (jcount - 1) + 1), :],
                        )
                        nc.scalar.dma_start(
                            n2g[ds(jstart, jcount), tens, :],
                            x_sb[ds(0, jcount, 32), :],
                        )

                # rsqrt: r = exp(-0.5*ln(n2+eps)); text half f32, vis bf16
                lnscr = scrpool.tile([G, 2, B], F32, tag="lnscr")
                nc.scalar.activation(
                    lnscr[:], n2g[:],
                    mybir.ActivationFunctionType.Ln, bias=eps_b[ds(0, G)],
                )
                r_g = rvpool.tile([G, B], F32, tag="rg")
                nc.scalar.activation(
                    r_g[:], lnscr[:, 0, :],
                    mybir.ActivationFunctionType.Exp, scale=-0.5,
                )
                r_gv = rvpool.tile([G, B], BF16, tag="rgv")
                nc.scalar.activation(
                    r_gv[:], lnscr[:, 1, :],
                    mybir.ActivationFunctionType.Exp, scale=-0.5,
                )

                # r_t -> [b-partition, (i, t)] via PE transposes (f32)
                rt_ps = ps_rt.tile([128, NB, G], F32, tag="rtps")
                for i in range(NB):
                    nc.tensor.matmul(
                        rt_ps[:, i, :],
                        r_g[:, ts(i, 128)],
                        identf[ds(0, G), ds(0, G)],
                        is_transpose=True,
                        skip_group_check=True,
                    )
                nc.vector.tensor_copy(r_t_sb[:, :, ds(g * G, G)], rt_ps[:])

                # vis r rows to partition 0 (partition_broadcast reads p0),
                # then per-token broadcast on GpSimd + normalize on DVE
                rv_flat = scrpool.tile([1, G, B], BF16, tag="rvflat")
                nc.scalar.dma_start(rv_flat[:], r_gv[:])
                for tl in range(G):
                    t = g * G + tl
                    rv_bc = rvpool.tile([128, B], BF16, tag="rvbc")
                    nc.gpsimd.partition_broadcast(
                        rv_bc[:], rv_flat[:, tl, :]
                    )
                    nc.vector.tensor_mul(
                        visT[:, t, 0, :], visT[:, t, 0, :], rv_bc[:]
                    )
                    nc.vector.tensor_mul(
                        visT[:, t, 1, :], visT[:, t, 1, :], rv_bc[:]
                    )

            # ---- phase B: dots, exp, cos*e, S/V PSUM accumulation ----
            # i = 0 is interleaved with phase A group emission; groups feed
            # the pipeline just-in-time.  Software pipeline depth 2.
            def make_phase_b(i):
                S_ps = ps_sv.tile([128, 512], F32, tag="Sps")
                V_ps = ps_sv.tile([128, 512], F32, tag="Vps")
                state = {"S": S_ps, "V": V_ps, "dots": {}}

                def emit_dots(t):
                    dots = ps_dots.tile([128, 512], F32, tag="dots")
                    nc.tensor.matmul(
                        dots[:], texT[:, t, 0, ts(i, 128)], visT[:, t, 0, :],
                        start=True, stop=False,
                    )
                    nc.tensor.matmul(
                        dots[:], texT[:, t, 1, ts(i, 128)], visT[:, t, 1, :],
                        start=False, stop=True,
                    )
                    state["dots"][t] = dots

                def emit_tail(t):
                    dots = state["dots"].pop(t)
                    e_t = epool.tile([128, 512], BF16, tag="e")
                    nc.scalar.activation(
                        e_t[:], dots[:],
                        mybir.ActivationFunctionType.Exp,
                        scale=r_t_sb[:, i, ds(t, 1)],
                    )
                    tmp_t = epool.tile([128, 512], BF16, tag="tmp")
                    nc.vector.scalar_tensor_tensor(
                        out=tmp_t[:],
                        in0=dots[:],
                        scalar=r_t_sb[:, i, ds(t, 1)],
                        in1=e_t[:],
                        op0=mybir.AluOpType.mult,
                        op1=mybir.AluOpType.mult,
                    )
                    nc.tensor.matmul(
                        state["S"][:], ident[:], e_t[:],
                        start=(t == 0), stop=(t == TLOC - 1),
                        skip_group_check=True,
                    )
                    nc.tensor.matmul(
                        state["V"][:], ident[:], tmp_t[:],
                        start=(t == 0), stop=(t == TLOC - 1),
                        skip_group_check=True,
                    )

                def finish(i=i):
                    nc.vector.tensor_copy(S_sb[:, i, :], state["S"][:])
                    nc.vector.tensor_copy(V_sb[:, i, :], state["V"][:])

                return emit_dots, emit_tail, finish

            DEPTH = 1
            streams = {}

            def b_steps(i, gg):
                """Emit group gg's 5 pipeline steps for b-tile i, fully
                drained (dots pool has only 2 banks)."""
                if i not in streams:
                    streams[i] = make_phase_b(i)
                emit_dots_i, emit_tail_i, finish_i = streams[i]
                pend = []
                for tl in range(G):
                    t = gg * G + tl
                    emit_dots_i(t)
                    pend.append(t)
                    if len(pend) > DEPTH:
                        emit_tail_i(pend.pop(0))
                while pend:
                    emit_tail_i(pend.pop(0))

            def finish_b(i):
                streams[i][2]()

            # phase A groups feed b-tiles 0 and 1, lagging 1 and 2 groups
            emit_A(0)
            emit_A(1); b_steps(0, 0)
            emit_A(2); b_steps(0, 1); b_steps(1, 0)
            emit_A(3); b_steps(0, 2); b_steps(1, 1)
            emit_A(4); b_steps(0, 3); b_steps(1, 2)
            b_steps(0, 4); b_steps(1, 3)
            finish_b(0)
            b_steps(1, 4)
            finish_b(1)

            def run_stream_i(i):
                for gg in range(NG):
                    b_steps(i, gg)
                finish_b(i)

            # ---- AllReduce S/V in two fp16 chunks (i-pairs), overlapping the
            # first chunk's collective with the second half of phase B ----
            cc_in = dpool.tile([2, 2, 128, 2 * 512], F16, tag="cc_in")
            cc_out0 = dpool.tile(
                [2, 128, 2 * 512], F16, tag="cc_out0", addr_space="Shared"
            )
            cc_out1 = dpool.tile(
                [2, 128, 2 * 512], F16, tag="cc_out1", addr_space="Shared"
            )
            cc_outs = (cc_out0, cc_out1)

            def ar_chunk(ch):
                isl = ds(2 * ch, 2)
                nc.sync.dma_start(
                    cc_in[ch, 0], S_sb[:, isl, :].rearrange("p i c -> p (i c)")
                )
                nc.sync.dma_start(
                    cc_in[ch, 1], V_sb[:, isl, :].rearrange("p i c -> p (i c)")
                )
                nc.gpsimd.collective_compute(
                    "AllReduce",
                    mybir.AluOpType.add,
                    replica_groups=[list(range(NCORES))],
                    ins=[cc_in[ch].opt()],
                    outs=[cc_outs[ch][:].opt()],
                )
                nc.sync.dma_start(
                    S_sb[:, isl, :].rearrange("p i c -> p (i c)"), cc_outs[ch][0]
                )
                nc.sync.dma_start(
                    V_sb[:, isl, :].rearrange("p i c -> p (i c)"), cc_outs[ch][1]
                )

            ar_chunk(0)
            run_stream_i(2)
            run_stream_i(3)
            ar_chunk(1)

            # ---- final scalar loss, split per AR chunk so chunk-0 finals
            # overlap the chunk-1 collective ----
            scr2 = finpool.tile([128, NB, 512], F32, tag="scr")
            sim = V_sb  # sim = V/S computed in place over V
            diag_p = finpool.tile([128, 2], F32, tag="diagp")
            rowsum = finpool.tile([128, NB], F32, tag="rowsum")
            col_ps = ps_sv.tile([1, 512], F32, tag="Sps")

            def finals_chunk(ch):
                isl = ds(2 * ch, 2)
                # pad correction: each global pad slice added exp(0)=1 to S
                nc.vector.tensor_scalar_add(
                    S_sb[:, isl, :], S_sb[:, isl, :], float(-NPAD)
                )
                nc.scalar.activation(
                    scr2[:, isl, :], S_sb[:, isl, :],
                    mybir.ActivationFunctionType.Ln,
                )
                nc.scalar.activation(
                    scr2[:, isl, :], scr2[:, isl, :],
                    mybir.ActivationFunctionType.Exp, scale=-1.0,
                )
                nc.vector.tensor_mul(
                    sim[:, isl, :], V_sb[:, isl, :], scr2[:, isl, :]
                )
                nc.vector.scalar_tensor_tensor(
                    out=scr2[:, isl, :],
                    in0=sim[:, isl, :],
                    scalar=1.0,
                    in1=diag_mask[:, isl, :],
                    op0=mybir.AluOpType.mult,
                    op1=mybir.AluOpType.mult,
                    accum_out=diag_p[:, ds(ch, 1)],
                )
                for i in range(2 * ch, 2 * ch + 2):
                    nc.scalar.activation(
                        scr2[:, i, :], sim[:, i, :],
                        mybir.ActivationFunctionType.Exp,
                        scale=1.0 / TEMP,
                        accum_out=rowsum[:, ds(i, 1)],
                    )
                for i in range(2 * ch, 2 * ch + 2):
                    nc.tensor.matmul(
                        col_ps[:], ones[:], scr2[:, i, :],
                        start=(i == 0), stop=(i == NB - 1),
                        skip_group_check=True,
                    )

            finals_chunk(0)
            finals_chunk(1)

            lse_row = finpool.tile([128, NB], F32, tag="lserow")
            nc.scalar.activation(
                lse_row[:], rowsum[:], mybir.ActivationFunctionType.Ln,
            )
            lse_col = finpool.tile([1, 512], F32, tag="lsecol")
            nc.scalar.activation(
                lse_col[:], col_ps[:], mybir.ActivationFunctionType.Ln,
            )
            csum = finpool.tile([1, 1], F32, tag="csum")
            nc.vector.reduce_sum(csum[:], lse_col[:], axis=mybir.AxisListType.X)
            red_ps = ps_sv.tile([1, 2], F32, tag="Vps")
            lse_row_red = finpool.tile([128, 1], F32, tag="lserr")
            nc.vector.reduce_sum(
                lse_row_red[:], lse_row[:], axis=mybir.AxisListType.X
            )
            diag_r = finpool.tile([128, 1], F32, tag="diagr")
            nc.vector.reduce_sum(diag_r[:], diag_p[:], axis=mybir.AxisListType.X)
            nc.tensor.matmul(
                red_ps[:, 0:1], ones[:], lse_row_red[:], start=True, stop=True
            )
            nc.tensor.matmul(
                red_ps[:, 1:2], ones[:], diag_r[:], start=True, stop=True
            )
            red_sb = finpool.tile([1, 2], F32, tag="redsb")
            nc.vector.tensor_copy(red_sb[:], red_ps[:])
            t_a = finpool.tile([1, 1], F32, tag="ta")
            nc.vector.tensor_add(t_a[:], red_sb[:, 0:1], csum[:])
            nc.vector.tensor_scalar_mul(t_a[:], t_a[:], 0.5 / B)
            t_b = finpool.tile([1, 1], F32, tag="tb")
            nc.vector.tensor_scalar_mul(
                t_b[:], red_sb[:, 1:2], 1.0 / (B * TEMP)
            )
            loss_t = finpool.tile([1, 1], F32, tag="loss")
            nc.vector.tensor_sub(loss_t[:], t_a[:], t_b[:])
            nc.sync.dma_start(out.ap(), loss_t[:])

    nc.compile()
    return nc


_CACHE = {}


def _get_program():
    if "nc" not in _CACHE:
        _CACHE["nc"] = _build_program()
    return _CACHE["nc"]


def _prep_core_inputs(text: np.ndarray, vis: np.ndarray):
    """Pad T, cast bf16, transpose to [p=d%128, t, h=d//128, b] per core."""
    import ml_dtypes

    bf16 = ml_dtypes.bfloat16
    tp = np.zeros((B, TPAD, D), np.float32)
    vp = np.zeros((B, TPAD, D), np.float32)
    tp[:, :T] = text
    vp[:, :T] = vis
    tpb = tp.astype(bf16)
    vpb = vp.astype(bf16)

    in_maps = []
    for k in range(NCORES):
        sl = slice(k * TLOC, (k + 1) * TLOC)
        core = {}
        for name, arr in (("text", tpb[:, sl]), ("vis", vpb[:, sl])):
            # [b, t, d] -> [d, t, b] -> [h, p, t, b] -> [p, t, h, b]
            x = arr.transpose(2, 1, 0).reshape(2, 128, TLOC, B)
            core[name] = np.ascontiguousarray(x.transpose(1, 2, 0, 3))
        in_maps.append(core)
    return in_maps


def kernel(text_tokens: np.ndarray, visual_tokens: np.ndarray) -> np.ndarray:
    text = np.ascontiguousarray(np.asarray(text_tokens, dtype=np.float32))
    vis = np.ascontiguousarray(np.asarray(visual_tokens, dtype=np.float32))
    assert text.shape == (B, T, D) and vis.shape == (B, T, D)

    in_maps = _prep_core_inputs(text, vis)
    nc = _get_program()
    res = run_bass_kernel_spmd(nc, in_maps, core_ids=list(range(NCORES)))
    loss = np.float32(res.results[0]["out"].reshape(-1)[0])
    return np.asarray(loss, dtype=np.float32).reshape(())


# revision 29
# speedup vs baseline: 1.1793x; 1.0813x over previous
"""Trainium2 Bass kernel for token-level contrastive loss (CLIP-style with
softmax token pooling), distributed over 8 NeuronCores.

Strategy: shard the token axis T (196 -> padded 200 = 8 cores x 25 slices).
Each core computes, for its local t-slices, the full [B, B] partial softmax-
pooling sums S = sum_t exp(cos_t) and V = sum_t cos_t*exp(cos_t); these are
AllReduced, then every core redundantly computes the scalar bidirectional
InfoNCE loss (core 0's output is returned).

v2 pipeline: the host ships bf16 tokens already transposed to [d, t, b]
layout, so the device does no transposes at all.  Per core:
  - token norms: DVE squares + ones-column matmuls that land n2 in [t, b]
    PSUM tiles; rsqrt via Ln/Exp on ACT
  - r_t (text rsqrt) transposed to [b-partition, t] by tiny PE transposes,
    consumed directly from PSUM as the ACT exp scale
  - visual tokens normalized in-place by GpSimd (partition_broadcast of the
    rsqrt row + multiply), keeping DVE free
  - per (b-tile, t): bf16 matmul -> dots PSUM; exp on ScalarE with r_t
    folded into the activation scale; cos*e via one fused DVE stt; S and V
    accumulated across all 25 t in dedicated PSUM banks by identity-matmuls
  - S/V flushed once per b-tile, AllReduced, and the scalar loss computed
    redundantly on every core.
"""

import sys

sys.path.insert(0, "/opt/trn_rl_repo")

import numpy as np

import concourse.bass as bass
import concourse.mybir as mybir
import concourse.tile as tile
from concourse import bacc
from concourse.bass import ds, ts
from concourse.bass_utils import run_bass_kernel_spmd
from concourse.masks import make_identity

B = 512
T = 196
D = 256
NCORES = 8
TPAD = 200
TLOC = TPAD // NCORES  # 25
G = 5                  # t-slices per norm group
NG = TLOC // G         # 5 groups
NB = B // 128          # 4 b-tiles
NPAD = TPAD - T        # 4 zero pad slices globally
TEMP = 0.07

F32 = mybir.dt.float32
F16 = mybir.dt.float16
BF16 = mybir.dt.bfloat16


def _build_program():
    nc = bacc.Bacc(
        "TRN2",
        target_bir_lowering=False,
        debug=False,
        num_devices=NCORES,
    )
    # host-pretransposed: [p=d%128, t, h=d//128, b]
    text_in = nc.dram_tensor("text", [128, TLOC, 2, B], BF16, kind="ExternalInput")
    vis_in = nc.dram_tensor("vis", [128, TLOC, 2, B], BF16, kind="ExternalInput")
    out = nc.dram_tensor("out", [1, 1], F32, kind="ExternalOutput")

    # n2 rows are bank-packed by PE at partition bases {0,32,64}; triples of
    # tokens per PSUM tile, groups of 5 -> sub-triples (3, 2) per group.
    TRIPLES = ((0, 3), (3, 2))  # (start_tl, count) within a group

    with tile.TileContext(nc) as tc:
        with (
            tc.tile_pool(name="const", bufs=1) as cpool,
            tc.tile_pool(name="tok", bufs=1) as tokpool,
            tc.tile_pool(name="xsq", bufs=1) as xsqpool,
            tc.tile_pool(name="xsb", bufs=2) as xsbpool,
            tc.tile_pool(name="rsb", bufs=1) as rpool,
            tc.tile_pool(name="et", bufs=3) as epool,
            tc.tile_pool(name="acc", bufs=1) as accpool,
            tc.tile_pool(name="fin", bufs=1) as finpool,
            tc.tile_pool(name="rvp", bufs=2) as rvpool,
            tc.tile_pool(name="scr", bufs=1) as scrpool,
            tc.tile_pool(name="psn", bufs=1, space="PSUM") as ps_n2,
            tc.tile_pool(name="psr", bufs=1, space="PSUM") as ps_rt,
            tc.tile_pool(name="pssv", bufs=2, space="PSUM") as ps_sv,
            tc.tile_pool(name="psd", bufs=2, space="PSUM") as ps_dots,
            tc.tile_pool(name="dram", bufs=1, space="DRAM") as dpool,
        ):
            # ---- constants ----
            ident = cpool.tile([128, 128], BF16, tag="ident")
            make_identity(nc, ident[:])
            identf = cpool.tile([128, 128], F32, tag="identf")
            make_identity(nc, identf[:])
            ones_bf = cpool.tile([128, 1], BF16, tag="onesbf")
            nc.gpsimd.memset(ones_bf[:], 1.0)
            ones = cpool.tile([128, 1], F32, tag="ones")
            nc.gpsimd.memset(ones[:], 1.0)
            eps_b = cpool.tile([128, 1], F32, tag="epsb")
            nc.gpsimd.memset(eps_b[:], 1e-12)
            diag_mask = cpool.tile([128, NB, 512], BF16, tag="dmask")
            nc.gpsimd.memset(diag_mask[:], 0.0)
            nc.gpsimd.affine_select(
                out=diag_mask[:],
                in_=diag_mask[:],
                compare_op=mybir.AluOpType.not_equal,
                fill=1.0,
                base=0,
                pattern=[[-128, NB], [1, 512]],
                channel_multiplier=-1,
            )

            # ---- persistent SBUF tiles ----
            texT = tokpool.tile([128, TLOC, 2, B], BF16, tag="texT")
            visT = tokpool.tile([128, TLOC, 2, B], BF16, tag="visT")
            # n2 / r gathered as [tl(5), g, (text|vis), b]
            r_t_sb = rpool.tile([128, NB, TLOC], F32, tag="rtsb")
            S_sb = accpool.tile([128, NB, 512], F16, tag="S")
            V_sb = accpool.tile([128, NB, 512], F16, tag="V")

            # ---- input loads: all issued up-front on the sync queue so the
            # DMA engines stream continuously (nothing queued behind them) ----
            for g in range(NG):
                tg = ds(g * G, G)
                nc.sync.dma_start(texT[:, tg, :, :], text_in.ap()[:, tg, :, :])
                nc.sync.dma_start(visT[:, tg, :, :], vis_in.ap()[:, tg, :, :])

            # ---- phase A helper: norms, rsqrt, vis normalize for group g ----
            def emit_A(g):
                tg = ds(g * G, G)
                xsq = xsqpool.tile([128, G, 2, 2, 512], BF16, tag="xsq")
                nc.vector.tensor_mul(
                    xsq[:, :, :, 0, :], texT[:, tg, :, :], texT[:, tg, :, :]
                )
                nc.vector.tensor_mul(
                    xsq[:, :, :, 1, :], visT[:, tg, :, :], visT[:, tg, :, :]
                )

                # n2 rows -> PSUM, bank-packed 3 tokens at bases {0,32,64};
                # evict to SBUF (partition-preserving), then SBUF->SBUF DMAs
                # scatter rows onto [tl] partitions of n2g.
                n2g = scrpool.tile([G, 2, B], F32, tag="n2g")
                for tens in range(2):
                    for jstart, jcount in TRIPLES:
                        x_ps = ps_n2.tile([65, 512], F32, tag="xps")
                        for s in range(jcount):
                            tl = jstart + s
                            for h in range(2):
                                nc.tensor.matmul(
                                    x_ps[ds(32 * s, 1), :],
                                    ones_bf[:],
                                    xsq[:, tl, h, tens, :],
                                    start=(h == 0), stop=(h == 1),
                                    skip_group_check=True,
                                )
                        x_sb = xsbpool.tile([65, 512], F32, tag="xsb")
                        nc.vector.tensor_copy(
                            x_sb[ds(0, 32 * (jcount - 1) + 1), :],
                            x_ps[ds(0, 32 * (jcount - 1) + 1), :],
                        )
                        nc.scalar.dma_start(
                            n2g[ds(jstart, jcount), tens, :],
                            x_sb[ds(0, jcount, 32), :],
                        )

                # rsqrt: r = exp(-0.5*ln(n2+eps)); text half f32, vis bf16
                lnscr = scrpool.tile([G, 2, B], F32, tag="lnscr")
                nc.scalar.activation(
                    lnscr[:], n2g[:],
                    mybir.ActivationFunctionType.Ln, bias=eps_b[ds(0, G)],
                )
                r_g = rvpool.tile([G, B], F32, tag="rg")
                nc.scalar.activation(
                    r_g[:], lnscr[:, 0, :],
                    mybir.ActivationFunctionType.Exp, scale=-0.5,
                )
                r_gv = rvpool.tile([G, B], BF16, tag="rgv")
                nc.scalar.activation(
                    r_gv[:], lnscr[:, 1, :],
                    mybir.ActivationFunctionType.Exp, scale=-0.5,
                )

                # r_t -> [b-partition, (i, t)] via PE transposes (f32)
                rt_ps = ps_rt.tile([128, NB, G], F32, tag="rtps")
                for i in range(NB):
                    nc.tensor.matmul(
                        rt_ps[:, i, :],
                        r_g[:, ts(i, 128)],
                        identf[ds(0, G), ds(0, G)],
                        is_transpose=True,
                        skip_group_check=True,
                    )
                nc.vector.tensor_copy(r_t_sb[:, :, ds(g * G, G)], rt_ps[:])

                # vis r rows to partition 0 (partition_broadcast reads p0),
                # then per-token broadcast on GpSimd + normalize on DVE
                rv_flat = scrpool.tile([1, G, B], BF16, tag="rvflat")
                nc.scalar.dma_start(rv_flat[:], r_gv[:])
                for tl in range(G):
                    t = g * G + tl
                    rv_bc = rvpool.tile([128, B], BF16, tag="rvbc")
                    nc.gpsimd.partition_broadcast(
                        rv_bc[:], rv_flat[:, tl, :]
                    )
                    nc.vector.tensor_mul(
                        visT[:, t, 0, :], visT[:, t, 0, :], rv_bc[:]
                    )
                    nc.vector.tensor_mul(
                        visT[:, t, 1, :], visT[:, t, 1, :], rv_bc[:]
                    )

            # ---- phase B: dots, exp, cos*e, S/V PSUM accumulation ----
            # i = 0 is interleaved with phase A group emission; groups feed
            # the pipeline just-in-time.  Software pipeline depth 2.
            def make_phase_b(i):
                S_ps = ps_sv.tile([128, 512], F32, tag="Sps")
                V_ps = ps_sv.tile([128, 512], F32, tag="Vps")
                state = {"S": S_ps, "V": V_ps, "dots": {}}

                def emit_dots(t):
                    dots = ps_dots.tile([128, 512], F32, tag="dots")
                    nc.tensor.matmul(
                        dots[:], texT[:, t, 0, ts(i, 128)], visT[:, t, 0, :],
                        start=True, stop=False,
                    )
                    nc.tensor.matmul(
                        dots[:], texT[:, t, 1, ts(i, 128)], visT[:, t, 1, :],
                        start=False, stop=True,
                    )
                    state["dots"][t] = dots

                def emit_tail(t):
                    dots = state["dots"].pop(t)
                    e_t = epool.tile([128, 512], BF16, tag="e")
                    nc.scalar.activation(
                        e_t[:], dots[:],
                        mybir.ActivationFunctionType.Exp,
                        scale=r_t_sb[:, i, ds(t, 1)],
                    )
                    tmp_t = epool.tile([128, 512], BF16, tag="tmp")
                    nc.vector.scalar_tensor_tensor(
                        out=tmp_t[:],
                        in0=dots[:],
                        scalar=r_t_sb[:, i, ds(t, 1)],
                        in1=e_t[:],
                        op0=mybir.AluOpType.mult,
                        op1=mybir.AluOpType.mult,
                    )
                    nc.tensor.matmul(
                        state["S"][:], ident[:], e_t[:],
                        start=(t == 0), stop=(t == TLOC - 1),
                        skip_group_check=True,
                    )
                    nc.tensor.matmul(
                        state["V"][:], ident[:], tmp_t[:],
                        start=(t == 0), stop=(t == TLOC - 1),
                        skip_group_check=True,
                    )

                def finish(i=i):
                    nc.vector.tensor_copy(S_sb[:, i, :], state["S"][:])
                    nc.vector.tensor_copy(V_sb[:, i, :], state["V"][:])

                return emit_dots, emit_tail, finish

            DEPTH = 1
            streams = {}

            def b_steps(i, gg):
                """Emit group gg's 5 pipeline steps for b-tile i, fully
                drained (dots pool has only 2 banks)."""
                if i not in streams:
                    streams[i] = make_phase_b(i)
                emit_dots_i, emit_tail_i, finish_i = streams[i]
                pend = []
                for tl in range(G):
                    t = gg * G + tl
                    emit_dots_i(t)
                    pend.append(t)
                    if len(pend) > DEPTH:
                        emit_tail_i(pend.pop(0))
                while pend:
                    emit_tail_i(pend.pop(0))

            def finish_b(i):
                streams[i][2]()

            # phase A groups feed b-tiles 0 and 1, lagging 1 and 2 groups
            emit_A(0)
            emit_A(1); b_steps(0, 0)
            emit_A(2); b_steps(0, 1); b_steps(1, 0)
            emit_A(3); b_steps(0, 2); b_steps(1, 1)
            emit_A(4); b_steps(0, 3); b_steps(1, 2)
            b_steps(0, 4); b_steps(1, 3)
            finish_b(0)
            b_steps(1, 4)
            finish_b(1)

            def run_stream_i(i):
                for gg in range(NG):
                    b_steps(i, gg)
                finish_b(i)

            # ---- AllReduce S/V in two fp16 chunks (i-pairs), overlapping the
            # first chunk's collective with the second half of phase B ----
            cc_in = dpool.tile([2, 2, 128, 2 * 512], F16, tag="cc_in")
            cc_out0 = dpool.tile(
                [2, 128, 2 * 512], F16, tag="cc_out0", addr_space="Shared"
            )
            cc_out1 = dpool.tile(
                [2, 128, 2 * 512], F16, tag="cc_out1", addr_space="Shared"
            )
            cc_outs = (cc_out0, cc_out1)

            def ar_chunk(ch):
                isl = ds(2 * ch, 2)
                nc.sync.dma_start(
                    cc_in[ch, 0], S_sb[:, isl, :].rearrange("p i c -> p (i c)")
                )
                nc.sync.dma_start(
                    cc_in[ch, 1], V_sb[:, isl, :].rearrange("p i c -> p (i c)")
                )
                nc.gpsimd.collective_compute(
                    "AllReduce",
                    mybir.AluOpType.add,
                    replica_groups=[list(range(NCORES))],
                    ins=[cc_in[ch].opt()],
                    outs=[cc_outs[ch][:].opt()],
                )
                nc.sync.dma_start(
                    S_sb[:, isl, :].rearrange("p i c -> p (i c)"), cc_outs[ch][0]
                )
                nc.sync.dma_start(
                    V_sb[:, isl, :].rearrange("p i c -> p (i c)"), cc_outs[ch][1]
                )

            ar_chunk(0)
            run_stream_i(2)
            run_stream_i(3)
            ar_chunk(1)

            # ---- final scalar loss, split per AR chunk so chunk-0 finals
            # overlap the chunk-1 collective ----
            scr2 = finpool.tile([128, NB, 512], F32, tag="scr")
            sim = V_sb  # sim = V/S computed in place over V
            diag_p = finpool.tile([128, 2], F32, tag="diagp")
            rowsum = finpool.tile([128, NB], F32, tag="rowsum")
            col_ps = ps_sv.tile([1, 512], F32, tag="Sps")

            def finals_chunk(ch):
                isl = ds(2 * ch, 2)
                # pad correction: each global pad slice added exp(0)=1 to S
                nc.vector.tensor_scalar_add(
                    S_sb[:, isl, :], S_sb[:, isl, :], float(-NPAD)
                )
                nc.scalar.activation(
                    scr2[:, isl, :], S_sb[:, isl, :],
                    mybir.ActivationFunctionType.Ln,
                )
                nc.scalar.activation(
                    scr2[:, isl, :], scr2[:, isl, :],
                    mybir.ActivationFunctionType.Exp, scale=-1.0,
                )
                nc.vector.tensor_mul(
                    sim[:, isl, :], V_sb[:, isl, :], scr2[:, isl, :]
                )
                nc.vector.scalar_tensor_tensor(
                    out=scr2[:, isl, :],
                    in0=sim[:, isl, :],
                    scalar=1.0,
                    in1=diag_mask[:, isl, :],
                    op0=mybir.AluOpType.mult,
                    op1=mybir.AluOpType.mult,
                    accum_out=diag_p[:, ds(ch, 1)],
                )
                for i in range(2 * ch, 2 * ch + 2):
                    nc.scalar.activation(
                        scr2[:, i, :], sim[:, i, :],
                        mybir.ActivationFunctionType.Exp,
                        scale=1.0 / TEMP,
                        accum_out=rowsum[:, ds(i, 1)],
                    )
                for i in range(2 * ch, 2 * ch + 2):
                    nc.tensor.matmul(
                        col_ps[:], ones[:], scr2[:, i, :],
                        start=(i == 0), stop=(i == NB - 1),
                        skip_group_check=True,
                    )

            finals_chunk(0)
            finals_chunk(1)

            lse_row = finpool.tile([128, NB], F32, tag="lserow")
            nc.scalar.activation(
                lse_row[:], rowsum[:], mybir.ActivationFunctionType.Ln,
            )
            lse_col = finpool.tile([1, 512], F32, tag="lsecol")
            nc.scalar.activation(
                lse_col[:], col_ps[:], mybir.ActivationFunctionType.Ln,
            )
            csum = finpool.tile([1, 1], F32, tag="csum")
            nc.vector.reduce_sum(csum[:], lse_col[:], axis=mybir.AxisListType.X)
            red_ps = ps_sv.tile([1, 2], F32, tag="Vps")
            lse_row_red = finpool.tile([128, 1], F32, tag="lserr")
            nc.vector.reduce_sum(
                lse_row_red[:], lse_row[:], axis=mybir.AxisListType.X
            )
            diag_r = finpool.tile([128, 1], F32, tag="diagr")
            nc.vector.reduce_sum(diag_r[:], diag_p[:], axis=mybir.AxisListType.X)
            nc.tensor.matmul(
                red_ps[:, 0:1], ones[:], lse_row_red[:], start=True, stop=True
            )
            nc.tensor.matmul(
                red_ps[:, 1:2], ones[:], diag_r[:], start=True, stop=True
            )
            red_sb = finpool.tile([1, 2], F32, tag="redsb")
            nc.vector.tensor_copy(red_sb[:], red_ps[:])
            t_a = finpool.tile([1, 1], F32, tag="ta")
            nc.vector.tensor_add(t_a[:], red_sb[:, 0:1], csum[:])
            nc.vector.tensor_scalar_mul(t_a[:], t_a[:], 0.5 / B)
            t_b = finpool.tile([1, 1], F32, tag="tb")
            nc.vector.tensor_scalar_mul(
                t_b[:], red_sb[:, 1:2], 1.0 / (B * TEMP)
            )
            loss_t = finpool.tile([1, 1], F32, tag="loss")
            nc.vector.tensor_sub(loss_t[:], t_a[:], t_b[:])
            nc.sync.dma_start(out.ap(), loss_t[:])

    nc.compile()
    return nc


_CACHE = {}


def _get_program():
    if "nc" not in _CACHE:
        _CACHE["nc"] = _build_program()
    return _CACHE["nc"]


def _prep_core_inputs(text: np.ndarray, vis: np.ndarray):
    """Pad T, cast bf16, transpose to [p=d%128, t, h=d//128, b] per core."""
    import ml_dtypes

    bf16 = ml_dtypes.bfloat16
    tp = np.zeros((B, TPAD, D), np.float32)
    vp = np.zeros((B, TPAD, D), np.float32)
    tp[:, :T] = text
    vp[:, :T] = vis
    tpb = tp.astype(bf16)
    vpb = vp.astype(bf16)

    in_maps = []
    for k in range(NCORES):
        sl = slice(k * TLOC, (k + 1) * TLOC)
        core = {}
        for name, arr in (("text", tpb[:, sl]), ("vis", vpb[:, sl])):
            # [b, t, d] -> [d, t, b] -> [h, p, t, b] -> [p, t, h, b]
            x = arr.transpose(2, 1, 0).reshape(2, 128, TLOC, B)
            core[name] = np.ascontiguousarray(x.transpose(1, 2, 0, 3))
        in_maps.append(core)
    return in_maps


def kernel(text_tokens: np.ndarray, visual_tokens: np.ndarray) -> np.ndarray:
    text = np.ascontiguousarray(np.asarray(text_tokens, dtype=np.float32))
    vis = np.ascontiguousarray(np.asarray(visual_tokens, dtype=np.float32))
    assert text.shape == (B, T, D) and vis.shape == (B, T, D)

    in_maps = _prep_core_inputs(text, vis)
    nc = _get_program()
    res = run_bass_kernel_spmd(nc, in_maps, core_ids=list(range(NCORES)))
    loss = np.float32(res.results[0]["out"].reshape(-1)[0])
    return np.asarray(loss, dtype=np.float32).reshape(())
